# revision 11
# baseline (speedup 1.0000x reference)
"""Trainium2 Bass kernel for the nn_Aggregate GNN message-passing problem.

Computation (see reference):
    keep = (A > 0) limited to the first `neibor_num` set entries per row
    nb_mean = (keep @ X) / max(cnt, 1)
    out = leaky_relu(X @ W_line.T + b_line)
        + where(cnt > 0, leaky_relu(nb_mean @ W_nb.T + b_nb), 0)

Sharding: rows of A / output rows are split across 8 cores (1024 rows each).
No collectives are needed: each core gets its A row-block (transposed), its
X row-block (transposed), the shared X head rows, and the weights.

Key structural fact exploited: `keep` zeroes every set bit after the
`neibor_num`-th, so only the first C columns of A can contribute, where C
bounds the column position of the nn-th set bit over all rows.  The host
verifies exactly (cheaply) that every row reaches `neibor_num` set bits
within the first C=256 columns; in that case cnt == neibor_num for every
row and the kernel contracts over 256 neighbor candidates instead of 8192.
If the check fails (it cannot for the target input distribution), a numpy
fallback computes the exact reference semantics.

Device pipeline per core (rows R=1024, C=256, Cin=Cout=512):
  1. mask:    mbT[j, r] = (A[r, j] > 0) via DVE is_gt on the transposed
              A block (int32 -> f32 0/1).
  2. prefix:  cumT = LTRI.T @ mbT per 128-column chunk (+ ONES.T @ mbT of
              earlier chunks) gives the inclusive prefix count of set bits
              along the row, in transposed layout, on the PE.
  3. keepT = (cumT <= nn) * mbT                      (one fused DVE op)
  4. nb_sumT = X_head.T-contract keepT               (PE, fp32r)
  5. xj = leaky(nb_sumT.T @ (W_nb.T/nn) + b_nb)      (PE + ACT)
     xi = leaky(X_blk @ W_line.T + b_line)           (PE + ACT)
     out = xi + xj                                   (DVE)
Biases are added with k=1 matmuls (ones-row x bias-row) into the same PSUM
accumulation group.  All matmuls use fp32r (single-pass FP22) precision.
"""

import numpy as np

NCORES = 8
N = 8192
CIN = 512
COUT = 512
R = N // NCORES          # rows per core
C = 256                  # neighbor-candidate column window
KC = C // 128            # 128-col chunks of the window
MC = CIN // 128          # 128-row chunks of the feature dim
RT = R // 128            # 128-row output tiles per core
NEG_SLOPE = 0.01         # jax.nn.leaky_relu default

_nc_cache = {}
LAST_RESULT = None       # BassKernelResults of the most recent device run
SIM_SAFE = False         # CoreSim lacks Lrelu; True swaps in a Relu decomposition


def _build_nc(nn: int):
    import concourse.bass as bass
    import concourse.bacc as bacc
    import concourse.mybir as mybir
    import concourse.tile as tile

    F32 = mybir.dt.float32
    F32R = mybir.dt.float32r
    I32 = mybir.dt.int32
    AF = mybir.ActivationFunctionType
    OP = mybir.AluOpType

    nc = bacc.Bacc("TRN2", target_bir_lowering=False, debug=False)

    at_d = nc.dram_tensor("at", [C, R], I32, kind="ExternalInput")
    xh_d = nc.dram_tensor("xh", [C, CIN], F32R, kind="ExternalInput")
    xt_d = nc.dram_tensor("xt", [CIN, R], F32R, kind="ExternalInput")
    wnbt_d = nc.dram_tensor("wnbt", [CIN, COUT], F32R, kind="ExternalInput")
    wlt_d = nc.dram_tensor("wlt", [CIN, COUT], F32R, kind="ExternalInput")
    bnb_d = nc.dram_tensor("bnb", [1, COUT], F32R, kind="ExternalInput")
    bl_d = nc.dram_tensor("bl", [1, COUT], F32R, kind="ExternalInput")
    ltri_d = nc.dram_tensor("ltri", [128, 128], F32R, kind="ExternalInput")
    onesm_d = nc.dram_tensor("onesm", [128, 128], F32R, kind="ExternalInput")
    out_d = nc.dram_tensor("out", [R, COUT], F32, kind="ExternalOutput")

    with tile.TileContext(nc) as tc:
        with (
            tc.tile_pool(name="const", bufs=1) as constp,
            tc.tile_pool(name="mask", bufs=1) as maskp,
            tc.tile_pool(name="work", bufs=3) as workp,
            tc.tile_pool(name="psum", bufs=2, space=bass.MemorySpace.PSUM) as psump,
        ):
            ltri = constp.tile([128, 128], F32R, name="ltri_sb")
            nc.sync.dma_start(ltri[:], ltri_d[:])
            ones = constp.tile([128, 128], F32R, name="ones_sb")
            nc.sync.dma_start(ones[:], onesm_d[:])
            bnb = constp.tile([1, COUT], F32R, name="bnb_sb")
            nc.sync.dma_start(bnb[:], bnb_d[:])
            bl = constp.tile([1, COUT], F32R, name="bl_sb")
            nc.sync.dma_start(bl[:], bl_d[:])

            xh = []
            for t in range(KC):
                xh_t = constp.tile([128, CIN], F32R, name=f"xh{t}")
                nc.sync.dma_start(xh_t[:], xh_d[t * 128:(t + 1) * 128, :])
                xh.append(xh_t)
            wnbt = []
            wlt = []
            xt = []
            for m in range(MC):
                w1 = constp.tile([128, COUT], F32R, name=f"wnbt{m}")
                nc.sync.dma_start(w1[:], wnbt_d[m * 128:(m + 1) * 128, :])
                wnbt.append(w1)
                w2 = constp.tile([128, COUT], F32R, name=f"wlt{m}")
                nc.sync.dma_start(w2[:], wlt_d[m * 128:(m + 1) * 128, :])
                wlt.append(w2)
                x1 = constp.tile([128, R], F32R, name=f"xt{m}")
                nc.sync.dma_start(x1[:], xt_d[m * 128:(m + 1) * 128, :])
                xt.append(x1)

            # 1. A block -> f32 0/1 mask, transposed layout [col, row]
            mb = []
            for t in range(KC):
                at_t = maskp.tile([128, R], I32, name=f"at{t}")
                nc.sync.dma_start(at_t[:], at_d[t * 128:(t + 1) * 128, :])
                mb_t = maskp.tile([128, R], F32R, name=f"mb{t}")
                nc.vector.tensor_scalar(mb_t[:], at_t[:], 0, None, op0=OP.is_gt)
                mb.append(mb_t)

            # 2+3. prefix count along the row (PE) -> keep mask (DVE)
            keep = []
            for t in range(KC):
                keep_t = maskp.tile([128, R], F32R, name=f"keep{t}")
                keep.append(keep_t)
            for t in range(KC):
                for h in range(R // 512):
                    sl = slice(h * 512, (h + 1) * 512)
                    cum = psump.tile([128, 512], F32, name="cum")
                    for s in range(t + 1):
                        nc.tensor.matmul(
                            cum[:],
                            ltri[:] if s == t else ones[:],
                            mb[s][:, sl],
                            start=(s == 0),
                            stop=(s == t),
                        )
                    # keep = (cum <= nn) * mb
                    nc.vector.scalar_tensor_tensor(
                        keep[t][:, sl], cum[:], float(nn), mb[t][:, sl],
                        op0=OP.is_le, op1=OP.mult,
                    )

            # 4. nb_sumT[c, r] = sum_k X[k, c] * keep[k, r]  (then pre-scaled
            #    weights absorb the 1/nn mean factor)
            nbm = []
            for m in range(MC):
                nbm_m = maskp.tile([128, R], F32R, name=f"nbm{m}")
                nbm.append(nbm_m)
            for m in range(MC):
                for h in range(R // 512):
                    sl = slice(h * 512, (h + 1) * 512)
                    ps = psump.tile([128, 512], F32, name="psnb")
                    for t in range(KC):
                        nc.tensor.matmul(
                            ps[:],
                            xh[t][:, m * 128:(m + 1) * 128],
                            keep[t][:, sl],
                            start=(t == 0),
                            stop=(t == KC - 1),
                        )
                    nc.scalar.activation(nbm[m][:, sl], ps[:], AF.Copy)

            # 5. two linears + leaky relu + add, per 128-row output tile
            def leaky(ps, out_sb):
                if SIM_SAFE:
                    t = workp.tile([128, COUT], F32, name="lrt")
                    nc.scalar.activation(t[:], ps[:], AF.Relu,
                                         scale=1.0 - NEG_SLOPE)
                    nc.vector.scalar_tensor_tensor(
                        out_sb[:], ps[:], NEG_SLOPE, t[:],
                        op0=OP.mult, op1=OP.add)
                else:
                    nc.scalar.activation(out_sb[:], ps[:], AF.Lrelu,
                                         alpha=NEG_SLOPE)

            for r in range(RT):
                rsl = slice(r * 128, (r + 1) * 128)
                psj = psump.tile([128, COUT], F32, name="psj")
                for m in range(MC):
                    nc.tensor.matmul(
                        psj[:], nbm[m][:, rsl], wnbt[m][:],
                        start=(m == 0), stop=False,
                    )
                nc.tensor.matmul(psj[:], ones[:1, :], bnb[:], start=False, stop=True)
                xj = workp.tile([128, COUT], F32, name="xj")
                leaky(psj, xj)

                psi = psump.tile([128, COUT], F32, name="psi")
                for m in range(MC):
                    nc.tensor.matmul(
                        psi[:], xt[m][:, rsl], wlt[m][:],
                        start=(m == 0), stop=False,
                    )
                nc.tensor.matmul(psi[:], ones[:1, :], bl[:], start=False, stop=True)
                xi = workp.tile([128, COUT], F32, name="xi")
                leaky(psi, xi)

                ot = workp.tile([128, COUT], F32, name="ot")
                nc.vector.tensor_tensor(ot[:], xi[:], xj[:], op=OP.add)
                nc.sync.dma_start(out_d[rsl, :], ot[:])

    nc.compile()
    return nc


def _get_nc(nn: int):
    if nn not in _nc_cache:
        _nc_cache[nn] = _build_nc(nn)
    return _nc_cache[nn]


def _numpy_fallback(X, A, W_nb, b_nb, W_line, b_line, nn):
    def leaky(x):
        return np.where(x >= 0, x, NEG_SLOPE * x)

    Ab = A > 0
    keep = Ab & (np.cumsum(Ab.astype(np.int64), axis=1) <= nn)
    cnt = keep.sum(axis=1, keepdims=True).astype(X.dtype)
    nb_sum = keep.astype(X.dtype) @ X
    nb_mean = nb_sum / np.maximum(cnt, 1.0)
    xj = leaky(nb_mean @ W_nb.T + b_nb)
    xi = leaky(X @ W_line.T + b_line)
    return (xi + np.where(cnt > 0, xj, 0.0)).astype(np.float32)


def build_in_maps(X, A, W_nb, b_nb, W_line, b_line, nn):
    """Shard the full inputs into one input map per core."""
    ATall = np.ascontiguousarray(A[:, :C].T.astype(np.int32))        # [C, N]
    XTall = np.ascontiguousarray(X.T)                                # [CIN, N]
    xh = np.ascontiguousarray(X[:C, :])                              # [C, CIN]
    wnbt = np.ascontiguousarray(W_nb.T) * np.float32(1.0 / nn)
    wlt = np.ascontiguousarray(W_line.T)
    bnb = np.ascontiguousarray(b_nb.reshape(1, COUT))
    bl = np.ascontiguousarray(b_line.reshape(1, COUT))
    ltri = np.triu(np.ones((128, 128), dtype=np.float32))            # [k<=j]
    onesm = np.ones((128, 128), dtype=np.float32)
    in_maps = []
    for c in range(NCORES):
        rows = slice(c * R, (c + 1) * R)
        in_maps.append({
            "at": np.ascontiguousarray(ATall[:, rows]),
            "xh": xh,
            "xt": np.ascontiguousarray(XTall[:, rows]),
            "wnbt": wnbt,
            "wlt": wlt,
            "bnb": bnb,
            "bl": bl,
            "ltri": ltri,
            "onesm": onesm,
        })
    return in_maps


def kernel(**inputs) -> np.ndarray:
    global LAST_RESULT
    X = np.ascontiguousarray(np.asarray(inputs["X"], dtype=np.float32))
    A = np.ascontiguousarray(np.asarray(inputs["A"], dtype=np.int32))
    W_nb = np.asarray(inputs["W_nb"], dtype=np.float32)
    b_nb = np.asarray(inputs["b_nb"], dtype=np.float32)
    W_line = np.asarray(inputs["W_line"], dtype=np.float32)
    b_line = np.asarray(inputs["b_line"], dtype=np.float32)
    nn = int(np.asarray(inputs["neibor_num"]))

    # Fast path requires: every row reaches nn set bits within the first C
    # columns (=> keep-mask confined to [:, :C] and cnt == nn > 0 per row).
    fast = (
        X.shape == (N, CIN) and A.shape == (N, N) and 1 <= nn <= C
        and int(np.count_nonzero(A[:, :C] > 0, axis=1).min()) >= nn
    )
    if not fast:
        return _numpy_fallback(X, A, W_nb, b_nb, W_line, b_line, nn)

    from concourse.bass_utils import run_bass_kernel_spmd

    in_maps = build_in_maps(X, A, W_nb, b_nb, W_line, b_line, nn)
    nc = _get_nc(nn)
    res = run_bass_kernel_spmd(nc, in_maps, core_ids=list(range(NCORES)))
    LAST_RESULT = res
    return np.concatenate([r["out"] for r in res.results], axis=0)


if __name__ == "__main__":
    rng = np.random.default_rng(0)
    X = rng.standard_normal((N, CIN), dtype=np.float32)
    A = (rng.random((N, N)) < 0.5).astype(np.int32)
    W_nb = rng.standard_normal((COUT, CIN), dtype=np.float32) * 0.04
    b_nb = rng.standard_normal(COUT, dtype=np.float32) * 0.04
    W_line = rng.standard_normal((COUT, CIN), dtype=np.float32) * 0.04
    b_line = rng.standard_normal(COUT, dtype=np.float32) * 0.04
    out = kernel(X=X, A=A, W_nb=W_nb, b_nb=b_nb, W_line=W_line,
                 b_line=b_line, neibor_num=64)
    exp = _numpy_fallback(X, A, W_nb, b_nb, W_line, b_line, 64)
    err = np.abs(out - exp).max() / np.abs(exp).max()
    print("self-test rel err:", err)


# revision 12
# speedup vs baseline: 1.2065x; 1.2065x over previous
"""Trainium2 Bass kernel for the nn_Aggregate GNN message-passing problem.

Computation (see reference):
    keep = (A > 0) limited to the first `neibor_num` set entries per row
    nb_mean = (keep @ X) / max(cnt, 1)
    out = leaky_relu(X @ W_line.T + b_line)
        + where(cnt > 0, leaky_relu(nb_mean @ W_nb.T + b_nb), 0)

Sharding: rows of A / output rows are split across 8 cores (1024 rows each).
No collectives are needed: each core gets its A row-block (transposed), its
X row-block (transposed), the shared X head rows, and the weights.

Key structural fact exploited: `keep` zeroes every set bit after the
`neibor_num`-th, so only the first C columns of A can contribute, where C
bounds the column position of the nn-th set bit over all rows.  The host
verifies exactly (cheaply) that every row reaches `neibor_num` set bits
within the first C=256 columns; in that case cnt == neibor_num for every
row and the kernel contracts over 256 neighbor candidates instead of 8192.
If the check fails (it cannot for the target input distribution), a numpy
fallback computes the exact reference semantics.

Device pipeline per core (rows R=1024, C=256, Cin=Cout=512):
  1. mask:    mbT[j, r] = (A[r, j] > 0) via DVE is_gt on the transposed
              A block (uint8 -> bf16 0/1; exact).
  2. prefix:  cumT = LTRI.T @ mbT per 128-column chunk (+ ONES.T @ mbT of
              earlier chunks) gives the inclusive prefix count of set bits
              along the row, in transposed layout, on the PE (bf16 inputs,
              fp32 accumulation; counts <= 256 so exact).
  3. keepT = (cumT <= nn) * mbT                      (one fused DVE op)
  4. nb_sumT = X_head.T-contract keepT               (PE, fp32r)
  5. xj = leaky(nb_sumT.T @ (W_nb.T/nn) + b_nb)      (PE + ACT Lrelu)
     xi = leaky(X_blk @ W_line.T + b_line)           (PE + ACT Lrelu)
     out = xi + xj                                   (DVE)
Biases are added with k=1 matmuls (ones-row x bias-row) into the same PSUM
accumulation group.  Flop-carrying matmuls use fp32r (single-pass FP22).

DMA split: latency-critical tensors (ltri/ones/at/xh/biases) ride the HW
DGE (nc.sync); the bulk stage-2 operands (xt, weights) ride the SW DGE
(nc.gpsimd) in parallel so the mask/neighbor pipeline starts immediately.
"""

import numpy as np

NCORES = 8
N = 8192
CIN = 512
COUT = 512
R = N // NCORES          # rows per core
C = 256                  # neighbor-candidate column window
KC = C // 128            # 128-col chunks of the window
MC = CIN // 128          # 128-row chunks of the feature dim
RT = R // 128            # 128-row output tiles per core
NEG_SLOPE = 0.01         # jax.nn.leaky_relu default

_nc_cache = {}
LAST_RESULT = None       # BassKernelResults of the most recent device run
SIM_SAFE = False         # CoreSim lacks Lrelu; True swaps in a Relu decomposition


def _build_nc(nn: int):
    import concourse.bass as bass
    import concourse.bacc as bacc
    import concourse.mybir as mybir
    import concourse.tile as tile

    F32 = mybir.dt.float32
    F32R = mybir.dt.float32r
    BF16 = mybir.dt.bfloat16
    U8 = mybir.dt.uint8
    AF = mybir.ActivationFunctionType
    OP = mybir.AluOpType

    nc = bacc.Bacc("TRN2", target_bir_lowering=False, debug=False)

    at_d = nc.dram_tensor("at", [C, R], U8, kind="ExternalInput")
    xh_d = nc.dram_tensor("xh", [C, CIN], F32R, kind="ExternalInput")
    xt_d = nc.dram_tensor("xt", [CIN, R], F32R, kind="ExternalInput")
    wnbt_d = nc.dram_tensor("wnbt", [CIN, COUT], F32R, kind="ExternalInput")
    wlt_d = nc.dram_tensor("wlt", [CIN, COUT], F32R, kind="ExternalInput")
    bnb_d = nc.dram_tensor("bnb", [1, COUT], F32R, kind="ExternalInput")
    bl_d = nc.dram_tensor("bl", [1, COUT], F32R, kind="ExternalInput")
    ltri_d = nc.dram_tensor("ltri", [128, 128], BF16, kind="ExternalInput")
    onesm_d = nc.dram_tensor("onesm", [128, 128], BF16, kind="ExternalInput")
    onesr_d = nc.dram_tensor("onesr", [1, 128], F32R, kind="ExternalInput")
    out_d = nc.dram_tensor("out", [R, COUT], F32, kind="ExternalOutput")

    with tile.TileContext(nc) as tc:
        with (
            tc.tile_pool(name="const", bufs=1) as constp,
            tc.tile_pool(name="mask", bufs=1) as maskp,
            tc.tile_pool(name="work", bufs=3) as workp,
            tc.tile_pool(name="psum", bufs=2, space=bass.MemorySpace.PSUM) as psump,
        ):
            # --- latency-critical loads (HW DGE) -------------------------
            ltri = constp.tile([128, 128], BF16, name="ltri_sb")
            nc.sync.dma_start(ltri[:], ltri_d[:])
            ones = constp.tile([128, 128], BF16, name="ones_sb")
            nc.sync.dma_start(ones[:], onesm_d[:])
            at = []
            for t in range(KC):
                at_t = maskp.tile([128, R], U8, name=f"at{t}")
                nc.sync.dma_start(at_t[:], at_d[t * 128:(t + 1) * 128, :])
                at.append(at_t)
            xh = []
            for t in range(KC):
                xh_t = constp.tile([128, CIN], F32R, name=f"xh{t}")
                nc.sync.dma_start(xh_t[:], xh_d[t * 128:(t + 1) * 128, :])
                xh.append(xh_t)
            onesr = constp.tile([1, 128], F32R, name="onesr_sb")
            nc.sync.dma_start(onesr[:], onesr_d[:])
            bnb = constp.tile([1, COUT], F32R, name="bnb_sb")
            nc.sync.dma_start(bnb[:], bnb_d[:])
            bl = constp.tile([1, COUT], F32R, name="bl_sb")
            nc.sync.dma_start(bl[:], bl_d[:])

            # --- bulk stage-2 operands (SW DGE, overlap with mask path) --
            wnbt = []
            wlt = []
            xt = []
            for m in range(MC):
                x1 = constp.tile([128, R], F32R, name=f"xt{m}")
                nc.gpsimd.dma_start(x1[:], xt_d[m * 128:(m + 1) * 128, :])
                xt.append(x1)
            for m in range(MC):
                w1 = constp.tile([128, COUT], F32R, name=f"wnbt{m}")
                nc.gpsimd.dma_start(w1[:], wnbt_d[m * 128:(m + 1) * 128, :])
                wnbt.append(w1)
                w2 = constp.tile([128, COUT], F32R, name=f"wlt{m}")
                nc.gpsimd.dma_start(w2[:], wlt_d[m * 128:(m + 1) * 128, :])
                wlt.append(w2)

            # 1. A block -> bf16 0/1 mask, transposed layout [col, row]
            mb = []
            for t in range(KC):
                mb_t = maskp.tile([128, R], BF16, name=f"mb{t}")
                nc.vector.tensor_scalar(mb_t[:], at[t][:], 0, None, op0=OP.is_gt)
                mb.append(mb_t)

            # 2+3. prefix count along the row (PE) -> keep mask (DVE)
            keep = []
            for t in range(KC):
                keep_t = maskp.tile([128, R], F32R, name=f"keep{t}")
                keep.append(keep_t)
            for t in range(KC):
                for h in range(R // 512):
                    sl = slice(h * 512, (h + 1) * 512)
                    cum = psump.tile([128, 512], F32, name="cum")
                    for s in range(t + 1):
                        nc.tensor.matmul(
                            cum[:],
                            ltri[:] if s == t else ones[:],
                            mb[s][:, sl],
                            start=(s == 0),
                            stop=(s == t),
                        )
                    # keep = (cum <= nn) * mb
                    nc.vector.scalar_tensor_tensor(
                        keep[t][:, sl], cum[:], float(nn), mb[t][:, sl],
                        op0=OP.is_le, op1=OP.mult,
                    )

            # 4. nb_sumT[c, r] = sum_k X[k, c] * keep[k, r]  (the pre-scaled
            #    weights absorb the 1/nn mean factor)
            nbm = []
            for m in range(MC):
                nbm_m = maskp.tile([128, R], F32R, name=f"nbm{m}")
                nbm.append(nbm_m)
            for m in range(MC):
                for h in range(R // 512):
                    sl = slice(h * 512, (h + 1) * 512)
                    ps = psump.tile([128, 512], F32, name="psnb")
                    for t in range(KC):
                        nc.tensor.matmul(
                            ps[:],
                            xh[t][:, m * 128:(m + 1) * 128],
                            keep[t][:, sl],
                            start=(t == 0),
                            stop=(t == KC - 1),
                        )
                    # PSUM -> SBUF copies split between ACT and DVE
                    if (m * 2 + h) % 2 == 0:
                        nc.scalar.activation(nbm[m][:, sl], ps[:], AF.Copy)
                    else:
                        nc.vector.tensor_copy(nbm[m][:, sl], ps[:])

            # 5. two linears + leaky relu + add, per 128-row output tile
            def leaky(ps, out_sb):
                if SIM_SAFE:
                    t = workp.tile([128, COUT], F32, name="lrt")
                    nc.scalar.activation(t[:], ps[:], AF.Relu,
                                         scale=1.0 - NEG_SLOPE)
                    nc.vector.scalar_tensor_tensor(
                        out_sb[:], ps[:], NEG_SLOPE, t[:],
                        op0=OP.mult, op1=OP.add)
                else:
                    nc.scalar.activation(out_sb[:], ps[:], AF.Lrelu,
                                         alpha=NEG_SLOPE)

            for r in range(RT):
                rsl = slice(r * 128, (r + 1) * 128)
                psj = psump.tile([128, COUT], F32, name="psj")
                for m in range(MC):
                    nc.tensor.matmul(
                        psj[:], nbm[m][:, rsl], wnbt[m][:],
                        start=(m == 0), stop=False,
                    )
                nc.tensor.matmul(psj[:], onesr[:], bnb[:], start=False, stop=True)
                xj = workp.tile([128, COUT], F32, name="xj")
                leaky(psj, xj)

                psi = psump.tile([128, COUT], F32, name="psi")
                for m in range(MC):
                    nc.tensor.matmul(
                        psi[:], xt[m][:, rsl], wlt[m][:],
                        start=(m == 0), stop=False,
                    )
                nc.tensor.matmul(psi[:], onesr[:], bl[:], start=False, stop=True)
                xi = workp.tile([128, COUT], F32, name="xi")
                leaky(psi, xi)

                ot = workp.tile([128, COUT], F32, name="ot")
                nc.vector.tensor_tensor(ot[:], xi[:], xj[:], op=OP.add)
                nc.sync.dma_start(out_d[rsl, :], ot[:])

    nc.compile()
    return nc


def _get_nc(nn: int):
    if nn not in _nc_cache:
        _nc_cache[nn] = _build_nc(nn)
    return _nc_cache[nn]


def _numpy_fallback(X, A, W_nb, b_nb, W_line, b_line, nn):
    def leaky(x):
        return np.where(x >= 0, x, NEG_SLOPE * x)

    Ab = A > 0
    keep = Ab & (np.cumsum(Ab.astype(np.int64), axis=1) <= nn)
    cnt = keep.sum(axis=1, keepdims=True).astype(X.dtype)
    nb_sum = keep.astype(X.dtype) @ X
    nb_mean = nb_sum / np.maximum(cnt, 1.0)
    xj = leaky(nb_mean @ W_nb.T + b_nb)
    xi = leaky(X @ W_line.T + b_line)
    return (xi + np.where(cnt > 0, xj, 0.0)).astype(np.float32)


def build_in_maps(X, A, W_nb, b_nb, W_line, b_line, nn):
    """Shard the full inputs into one input map per core."""
    import ml_dtypes

    ATall = np.ascontiguousarray((A[:, :C] > 0).T.astype(np.uint8))  # [C, N]
    XTall = np.ascontiguousarray(X.T)                                # [CIN, N]
    xh = np.ascontiguousarray(X[:C, :])                              # [C, CIN]
    wnbt = np.ascontiguousarray(W_nb.T) * np.float32(1.0 / nn)
    wlt = np.ascontiguousarray(W_line.T)
    bnb = np.ascontiguousarray(b_nb.reshape(1, COUT))
    bl = np.ascontiguousarray(b_line.reshape(1, COUT))
    ltri = np.triu(np.ones((128, 128), dtype=np.float32)).astype(ml_dtypes.bfloat16)
    onesm = np.ones((128, 128), dtype=ml_dtypes.bfloat16)
    onesr = np.ones((1, 128), dtype=np.float32)
    in_maps = []
    for c in range(NCORES):
        rows = slice(c * R, (c + 1) * R)
        in_maps.append({
            "at": np.ascontiguousarray(ATall[:, rows]),
            "xh": xh,
            "xt": np.ascontiguousarray(XTall[:, rows]),
            "wnbt": wnbt,
            "wlt": wlt,
            "bnb": bnb,
            "bl": bl,
            "ltri": ltri,
            "onesm": onesm,
            "onesr": onesr,
        })
    return in_maps


def kernel(**inputs) -> np.ndarray:
    global LAST_RESULT
    X = np.ascontiguousarray(np.asarray(inputs["X"], dtype=np.float32))
    A = np.ascontiguousarray(np.asarray(inputs["A"], dtype=np.int32))
    W_nb = np.asarray(inputs["W_nb"], dtype=np.float32)
    b_nb = np.asarray(inputs["b_nb"], dtype=np.float32)
    W_line = np.asarray(inputs["W_line"], dtype=np.float32)
    b_line = np.asarray(inputs["b_line"], dtype=np.float32)
    nn = int(np.asarray(inputs["neibor_num"]))

    # Fast path requires: every row reaches nn set bits within the first C
    # columns (=> keep-mask confined to [:, :C] and cnt == nn > 0 per row).
    fast = (
        X.shape == (N, CIN) and A.shape == (N, N) and 1 <= nn <= C
        and int(np.count_nonzero(A[:, :C] > 0, axis=1).min()) >= nn
    )
    if not fast:
        return _numpy_fallback(X, A, W_nb, b_nb, W_line, b_line, nn)

    from concourse.bass_utils import run_bass_kernel_spmd

    in_maps = build_in_maps(X, A, W_nb, b_nb, W_line, b_line, nn)
    nc = _get_nc(nn)
    res = run_bass_kernel_spmd(nc, in_maps, core_ids=list(range(NCORES)))
    LAST_RESULT = res
    return np.concatenate([r["out"] for r in res.results], axis=0)


if __name__ == "__main__":
    rng = np.random.default_rng(0)
    X = rng.standard_normal((N, CIN), dtype=np.float32)
    A = (rng.random((N, N)) < 0.5).astype(np.int32)
    W_nb = rng.standard_normal((COUT, CIN), dtype=np.float32) * 0.04
    b_nb = rng.standard_normal(COUT, dtype=np.float32) * 0.04
    W_line = rng.standard_normal((COUT, CIN), dtype=np.float32) * 0.04
    b_line = rng.standard_normal(COUT, dtype=np.float32) * 0.04
    out = kernel(X=X, A=A, W_nb=W_nb, b_nb=b_nb, W_line=W_line,
                 b_line=b_line, neibor_num=64)
    exp = _numpy_fallback(X, A, W_nb, b_nb, W_line, b_line, 64)
    err = np.abs(out - exp).max() / np.abs(exp).max()
    print("self-test rel err:", err)
